# revision 29
# baseline (speedup 1.0000x reference)
"""Grok1-style MoE (T=2048, H=1024, E=8, I=2048, top-2) on 8 Trainium2 cores.

Strategy (expert-parallel, per the sharding hint):
  - Host: compute the tiny router (x @ gate_w, tanh softcap, top-2, softmax)
    and dispatch tokens by expert assignment (the "all-to-all dispatch" step:
    with full inputs on the host, dispatch = gather per expert), packing the
    per-core shards in bf16 device-friendly layouts.
  - Device (SPMD, 1 expert per core), all matmuls bf16 (1 row/cycle, half
    the HBM traffic and LDWEIGHTS cost of fp32r):
      phase 1: gT = wg_e^T x^T ; uT = wu_e^T x^T     ([I, M], fp32 PSUM)
               act = gelu_tanh(gT) * uT              (bf16 SBUF)
      phase 2: yT[h, m] = p[m] * sum_i wd_e[i, h] act[i, m]
               (wd stationary, act moving -> no M-padding to 128 needed;
               the router probs are applied by the DVE in fp32 during the
               PSUM->SBUF drain, against a host-replicated [P, M] prob tile)
  - Host: combine = scatter-add per-expert y^T into [T, H].
"""

import numpy as np
import ml_dtypes

import concourse.mybir as mybir
import concourse.tile as tile
from concourse import bacc
from concourse.bass_utils import run_bass_kernel_spmd

T, H, E, I_DIM, TOPK = 2048, 1024, 8, 2048, 2
SOFTCAP = 30.0
P = 128
N_CORES = 8
KH = H // P      # 8 contraction tiles (phase 1)
NI = I_DIM // P  # 16 i tiles
NH = H // P      # 8 h tiles (phase 2 output partition tiles)

BF16 = ml_dtypes.bfloat16

_compiled = {}
LAST_RESULTS = None


def _m_chunks(M_PAD):
    """Split [0, M_PAD) into near-equal chunks <= 512 (PSUM bank: 512 fp32)."""
    n_chunks = max(1, -(-M_PAD // 512))
    base = -(-(M_PAD // n_chunks) // 8) * 8
    chunks, off = [], 0
    for _ in range(n_chunks):
        ln = min(base, M_PAD - off)
        chunks.append((off, ln))
        off += ln
    return [c for c in chunks if c[1] > 0]


def _build(M_PAD):
    f32 = mybir.dt.float32
    bf16 = mybir.dt.bfloat16
    chunks = _m_chunks(M_PAD)

    nc = bacc.Bacc("TRN2", target_bir_lowering=False, num_devices=N_CORES)
    # Host-packed layouts (all DMAs contiguous per partition):
    #   xt  [KH, P, M_PAD] : xt[k, p, m] = x_e[m, k*P+p]            (bf16)
    #   wg  [NI, P, KH*P]  : wg[it, p, k*P+i] = wg_e[k*P+p, it*P+i] (bf16)
    #   wu  same as wg
    #   wd  [I, H]         : natural layout                          (bf16)
    #   pb  [P, M_PAD]     : router prob per token, partition-replicated
    #   y   [NH, P, M_PAD] : y[h, p, m] = out_e[m, h*P+p]            (fp32)
    xt = nc.dram_tensor("xt", [KH, P, M_PAD], bf16, kind="ExternalInput")
    wg = nc.dram_tensor("wg", [NI, P, KH * P], bf16, kind="ExternalInput")
    wu = nc.dram_tensor("wu", [NI, P, KH * P], bf16, kind="ExternalInput")
    wd = nc.dram_tensor("wd", [I_DIM, H], bf16, kind="ExternalInput")
    pb = nc.dram_tensor("pb", [P, M_PAD], f32, kind="ExternalInput")
    y = nc.dram_tensor("y", [NH, P, M_PAD], f32, kind="ExternalOutput")

    with tile.TileContext(nc) as tc:
        with (
            tc.tile_pool(name="persist", bufs=1) as persist,
            tc.tile_pool(name="wtiles", bufs=3) as wtiles,
            tc.tile_pool(name="gelu", bufs=2) as gpool,
            tc.tile_pool(name="outs", bufs=3) as outs,
            tc.tile_pool(name="psum", bufs=2, space="PSUM") as psum,
        ):
            xt_sb = persist.tile([P, KH, M_PAD], bf16)
            wd_sb = persist.tile([P, NI, H], bf16)
            pb_sb = persist.tile([P, M_PAD], f32)
            acts = persist.tile([P, NI, M_PAD], bf16)
            warm = persist.tile([P, 512], bf16)

            def w_src(w, it):
                return w.ap()[it].rearrange("p (ko i) -> p ko i", i=P)

            wg_sbs, wu_sbs = {}, {}

            def load_w(it):
                wg_sbs[it] = wtiles.tile([P, KH, P], bf16, tag="wg", name=f"wg{it}")
                wu_sbs[it] = wtiles.tile([P, KH, P], bf16, tag="wu", name=f"wu{it}")
                nc.gpsimd.dma_start(wg_sbs[it][:], w_src(wg, it))
                nc.scalar.dma_start(wu_sbs[it][:], w_src(wu, it))

            # Startup: each dma_start costs ~0.7us of sequencer issue time
            # and a ring streams ~200 GB/s, so use few, big pieces and put
            # the x k-pair pieces on TWO rings (sync + scalar, interleaved
            # with wu0) so the first i-tile's accumulation can start ~2us
            # earlier.  wg tiles stream on gpsimd, wu tiles on scalar.
            # Warm up the PE p-state during the initial DMA wait: ~12 junk
            # matmuls on a zeroed scratch tile keep the tensor engine busy
            # from the preamble until the first real operands land, so the
            # real matmuls start at full clock instead of ramping through
            # the 0.65/1.2 GHz p-states.
            nc.scalar.memzero(warm[:])
            w_ps = psum.tile([P, 512], f32, tag="g")
            for _ in range(12):
                nc.tensor.matmul(w_ps[:], warm[:, :P], warm[:], start=True, stop=True)

            wg_sbs[0] = wtiles.tile([P, KH, P], bf16, tag="wg", name="wg0")
            wu_sbs[0] = wtiles.tile([P, KH, P], bf16, tag="wu", name="wu0")
            nc.gpsimd.dma_start(wg_sbs[0][:], w_src(wg, 0))

            def x_piece(eng, k0, k1):
                eng.dma_start(
                    xt_sb[:, k0:k1],
                    xt.ap()[k0:k1].rearrange("k p m -> p k m"),
                )

            x_piece(nc.sync, 0, 2)
            x_piece(nc.scalar, 2, 4)
            x_piece(nc.sync, 4, 6)
            nc.scalar.dma_start(wu_sbs[0][:], w_src(wu, 0))
            x_piece(nc.gpsimd, 6, 8)
            load_w(1)

            # Phase 1: gT/uT = wg^T xT / wu^T xT per i-tile; act = gelu(g)*u.
            # wd tile loads and the prob tile interleave on the sync ring
            # mid-phase (consumed only in phase 2).
            for it in range(NI):
                if it + 2 < NI:
                    load_w(it + 2)
                if 4 <= it < 12:
                    for j in range(2):
                        wd_it = 2 * (it - 4) + j
                        nc.sync.dma_start(
                            wd_sb[:, wd_it], wd.ap()[wd_it * P:(wd_it + 1) * P, :]
                        )
                if it == 12:
                    nc.sync.dma_start(pb_sb[:], pb.ap())
                wg_sb, wu_sb = wg_sbs.pop(it), wu_sbs.pop(it)

                for (m0, ml) in chunks:
                    g_ps = psum.tile([P, ml], f32, tag="g")
                    for k in range(KH):
                        nc.tensor.matmul(
                            g_ps[:],
                            wg_sb[:, k],
                            xt_sb[:, k, m0:m0 + ml],
                            start=(k == 0),
                            stop=(k == KH - 1),
                        )
                    u_ps = psum.tile([P, ml], f32, tag="u")
                    for k in range(KH):
                        nc.tensor.matmul(
                            u_ps[:],
                            wu_sb[:, k],
                            xt_sb[:, k, m0:m0 + ml],
                            start=(k == 0),
                            stop=(k == KH - 1),
                        )
                    t = gpool.tile([P, ml], bf16, tag="t")
                    nc.scalar.activation(
                        t[:], g_ps[:], mybir.ActivationFunctionType.Gelu_apprx_tanh
                    )
                    nc.vector.tensor_mul(acts[:, it, m0:m0 + ml], t[:], u_ps[:])

            # Phase 2: yT[h, m] = p[m] * sum_i wd[i, h] * act[i, m]
            # (wd tile stationary, act moving; prob applied during the
            # PSUM drain by the DVE).
            for h in range(NH):
                y_sb = outs.tile([P, M_PAD], f32, tag="y")
                for ci, (m0, ml) in enumerate(chunks):
                    y_ps = psum.tile([P, ml], f32, tag="y")
                    for it in range(NI):
                        nc.tensor.matmul(
                            y_ps[:],
                            wd_sb[:, it, h * P:(h + 1) * P],
                            acts[:, it, m0:m0 + ml],
                            start=(it == 0),
                            stop=(it == NI - 1),
                        )
                    nc.vector.tensor_mul(
                        y_sb[:, m0:m0 + ml], y_ps[:], pb_sb[:, m0:m0 + ml]
                    )
                eng = nc.sync if h % 2 == 0 else nc.gpsimd
                eng.dma_start(y.ap()[h], y_sb[:])

    nc.compile()
    return nc


def _pack_w(w_e):
    """[H, I] -> [NI, P, KH*P] bf16 with w[it, p, k*P+i] = w_e[k*P+p, it*P+i]."""
    w4 = w_e.reshape(KH, P, NI, P)
    return np.ascontiguousarray(
        w4.transpose(2, 1, 0, 3).reshape(NI, P, KH * P).astype(BF16)
    )


def kernel(hidden_states, gate_w, wg, wu, wd):
    global LAST_RESULTS
    x = np.ascontiguousarray(np.asarray(hidden_states, dtype=np.float32))
    gw = np.asarray(gate_w, dtype=np.float32)
    wg = np.asarray(wg, dtype=np.float32)
    wu = np.asarray(wu, dtype=np.float32)
    wd = np.asarray(wd, dtype=np.float32)

    # Router on host (part of the dispatch/sharding step).
    logits = np.tanh((x @ gw) / np.float32(SOFTCAP))
    top2 = np.argsort(-logits, axis=1, kind="stable")[:, :TOPK]  # [T, 2]
    v = np.take_along_axis(logits, top2, axis=1)                 # descending
    ex = np.exp(v - v[:, :1])
    pk = (ex / ex.sum(axis=1, keepdims=True)).astype(np.float32)  # [T, 2]

    token_ids, probs_e = [], []
    for e in range(E):
        mask = top2 == e
        rows = np.where(mask.any(axis=1))[0]
        kk = np.argmax(mask[rows], axis=1)
        token_ids.append(rows)
        probs_e.append(pk[rows, kk])

    n_max = max(len(r) for r in token_ids)
    M_PAD = max(64, -(-n_max // 8) * 8)

    nc = _compiled.get(M_PAD)
    if nc is None:
        nc = _build(M_PAD)
        _compiled[M_PAD] = nc

    in_maps = []
    for e in range(E):
        ids = token_ids[e]
        xe = np.zeros((M_PAD, H), np.float32)
        xe[: len(ids)] = x[ids]
        # [M_PAD, KH, P] -> [KH, P, M_PAD]
        xt_e = np.ascontiguousarray(
            xe.reshape(M_PAD, KH, P).transpose(1, 2, 0).astype(BF16)
        )
        pr = np.zeros((M_PAD,), np.float32)
        pr[: len(ids)] = probs_e[e]
        in_maps.append(
            {
                "xt": xt_e,
                "wg": _pack_w(wg[e]),
                "wu": _pack_w(wu[e]),
                "wd": np.ascontiguousarray(wd[e].astype(BF16)),
                "pb": np.ascontiguousarray(
                    np.broadcast_to(pr, (P, M_PAD)).copy()
                ),
            }
        )

    res = run_bass_kernel_spmd(nc, in_maps, core_ids=list(range(N_CORES)))
    LAST_RESULTS = res

    out = np.zeros((T, H), np.float32)
    for e in range(E):
        ids = token_ids[e]
        yT = res.results[e]["y"].reshape(H, M_PAD)          # [H, M_PAD]
        out[ids] += yT[:, : len(ids)].T
    return out


# revision 30
# speedup vs baseline: 1.0067x; 1.0067x over previous
"""Grok1-style MoE (T=2048, H=1024, E=8, I=2048, top-2) on 8 Trainium2 cores.

Strategy (expert-parallel, per the sharding hint):
  - Host: compute the tiny router (x @ gate_w, tanh softcap, top-2, softmax)
    and dispatch tokens by expert assignment (the "all-to-all dispatch" step:
    with full inputs on the host, dispatch = gather per expert), packing the
    per-core shards in bf16 device-friendly layouts.
  - Device (SPMD, 1 expert per core), all matmuls bf16 (1 row/cycle, half
    the HBM traffic and LDWEIGHTS cost of fp32r):
      phase 1: gT = wg_e^T x^T ; uT = wu_e^T x^T     ([I, M], fp32 PSUM)
               act = gelu_tanh(gT) * uT              (bf16 SBUF)
      phase 2: yT[h, m] = p[m] * sum_i wd_e[i, h] act[i, m]
               (wd stationary, act moving -> no M-padding to 128 needed;
               the router probs are applied by the DVE in fp32 during the
               PSUM->SBUF drain, against a host-replicated [P, M] prob tile)
  - Host: combine = scatter-add per-expert y^T into [T, H].
"""

import numpy as np
import ml_dtypes

import concourse.mybir as mybir
import concourse.tile as tile
from concourse import bacc
from concourse.bass_utils import run_bass_kernel_spmd

T, H, E, I_DIM, TOPK = 2048, 1024, 8, 2048, 2
SOFTCAP = 30.0
P = 128
N_CORES = 8
KH = H // P      # 8 contraction tiles (phase 1)
NI = I_DIM // P  # 16 i tiles
NH = H // P      # 8 h tiles (phase 2 output partition tiles)

BF16 = ml_dtypes.bfloat16

_compiled = {}
LAST_RESULTS = None


def _m_chunks(M_PAD):
    """Split [0, M_PAD) into near-equal chunks <= 512 (PSUM bank: 512 fp32)."""
    n_chunks = max(1, -(-M_PAD // 512))
    base = -(-(M_PAD // n_chunks) // 8) * 8
    chunks, off = [], 0
    for _ in range(n_chunks):
        ln = min(base, M_PAD - off)
        chunks.append((off, ln))
        off += ln
    return [c for c in chunks if c[1] > 0]


def _build(M_PAD):
    f32 = mybir.dt.float32
    bf16 = mybir.dt.bfloat16
    chunks = _m_chunks(M_PAD)

    nc = bacc.Bacc("TRN2", target_bir_lowering=False, num_devices=N_CORES)
    # Host-packed layouts (all DMAs contiguous per partition):
    #   xt  [KH, P, M_PAD] : xt[k, p, m] = x_e[m, k*P+p]            (bf16)
    #   wg  [NI, P, KH*P]  : wg[it, p, k*P+i] = wg_e[k*P+p, it*P+i] (bf16)
    #   wu  same as wg
    #   wd  [I, H]         : natural layout                          (bf16)
    #   pb  [P, M_PAD]     : router prob per token, partition-replicated
    #   y   [NH, P, M_PAD] : y[h, p, m] = out_e[m, h*P+p]            (fp32)
    xt = nc.dram_tensor("xt", [KH, P, M_PAD], bf16, kind="ExternalInput")
    wg = nc.dram_tensor("wg", [NI, P, KH * P], bf16, kind="ExternalInput")
    wu = nc.dram_tensor("wu", [NI, P, KH * P], bf16, kind="ExternalInput")
    wd = nc.dram_tensor("wd", [I_DIM, H], bf16, kind="ExternalInput")
    pb = nc.dram_tensor("pb", [P, M_PAD], f32, kind="ExternalInput")
    y = nc.dram_tensor("y", [NH, P, M_PAD], f32, kind="ExternalOutput")

    with tile.TileContext(nc) as tc:
        with (
            tc.tile_pool(name="persist", bufs=1) as persist,
            tc.tile_pool(name="wtiles", bufs=3) as wtiles,
            tc.tile_pool(name="gelu", bufs=2) as gpool,
            tc.tile_pool(name="outs", bufs=3) as outs,
            tc.tile_pool(name="psum", bufs=2, space="PSUM") as psum,
        ):
            xt_sb = persist.tile([P, KH, M_PAD], bf16)
            wd_sb = persist.tile([P, NI, H], bf16)
            pb_sb = persist.tile([P, M_PAD], f32)
            acts = persist.tile([P, NI, M_PAD], bf16)
            warm = persist.tile([P, 512], bf16)

            def w_src(w, it):
                return w.ap()[it].rearrange("p (ko i) -> p ko i", i=P)

            wg_sbs, wu_sbs = {}, {}

            def load_w(it):
                wg_sbs[it] = wtiles.tile([P, KH, P], bf16, tag="wg", name=f"wg{it}")
                wu_sbs[it] = wtiles.tile([P, KH, P], bf16, tag="wu", name=f"wu{it}")
                nc.gpsimd.dma_start(wg_sbs[it][:], w_src(wg, it))
                nc.scalar.dma_start(wu_sbs[it][:], w_src(wu, it))

            # Startup: each dma_start costs ~0.7us of sequencer issue time
            # and a ring streams ~200 GB/s, so use few, big pieces and put
            # the x k-pair pieces on TWO rings (sync + scalar, interleaved
            # with wu0) so the first i-tile's accumulation can start ~2us
            # earlier.  wg tiles stream on gpsimd, wu tiles on scalar.
            # Warm up the PE p-state during the initial DMA wait: ~12 junk
            # matmuls on a zeroed scratch tile keep the tensor engine busy
            # from the preamble until the first real operands land, so the
            # real matmuls start at full clock instead of ramping through
            # the 0.65/1.2 GHz p-states.
            nc.scalar.memzero(warm[:])
            w_ps = psum.tile([P, 512], f32, tag="g")
            for _ in range(12):
                nc.tensor.matmul(w_ps[:], warm[:, :P], warm[:], start=True, stop=True)

            wg_sbs[0] = wtiles.tile([P, KH, P], bf16, tag="wg", name="wg0")
            wu_sbs[0] = wtiles.tile([P, KH, P], bf16, tag="wu", name="wu0")
            nc.gpsimd.dma_start(wg_sbs[0][:], w_src(wg, 0))

            def x_piece(eng, k0, k1):
                eng.dma_start(
                    xt_sb[:, k0:k1],
                    xt.ap()[k0:k1].rearrange("k p m -> p k m"),
                )

            x_piece(nc.sync, 0, 2)
            x_piece(nc.scalar, 2, 4)
            x_piece(nc.sync, 4, 6)
            nc.scalar.dma_start(wu_sbs[0][:], w_src(wu, 0))
            x_piece(nc.gpsimd, 6, 8)
            load_w(1)

            # Phase 1: gT/uT = wg^T xT / wu^T xT per i-tile; act = gelu(g)*u.
            # wd tile loads and the prob tile interleave on the sync ring
            # mid-phase (consumed only in phase 2).
            for it in range(NI):
                if it + 2 < NI:
                    load_w(it + 2)
                if 4 <= it < 12:
                    for j in range(2):
                        wd_it = 2 * (it - 4) + j
                        nc.sync.dma_start(
                            wd_sb[:, wd_it], wd.ap()[wd_it * P:(wd_it + 1) * P, :]
                        )
                if it == 12:
                    nc.sync.dma_start(pb_sb[:], pb.ap())
                wg_sb, wu_sb = wg_sbs.pop(it), wu_sbs.pop(it)

                for (m0, ml) in chunks:
                    g_ps = psum.tile([P, ml], f32, tag="g")
                    for k in range(KH):
                        nc.tensor.matmul(
                            g_ps[:],
                            wg_sb[:, k],
                            xt_sb[:, k, m0:m0 + ml],
                            start=(k == 0),
                            stop=(k == KH - 1),
                        )
                    u_ps = psum.tile([P, ml], f32, tag="u")
                    for k in range(KH):
                        nc.tensor.matmul(
                            u_ps[:],
                            wu_sb[:, k],
                            xt_sb[:, k, m0:m0 + ml],
                            start=(k == 0),
                            stop=(k == KH - 1),
                        )
                    t = gpool.tile([P, ml], bf16, tag="t")
                    nc.scalar.activation(
                        t[:], g_ps[:], mybir.ActivationFunctionType.Gelu_apprx_tanh
                    )
                    nc.vector.tensor_mul(acts[:, it, m0:m0 + ml], t[:], u_ps[:])

            # Phase 2: yT[h, m] = p[m] * sum_i wd[i, h] * act[i, m]
            # (wd tile stationary, act moving; prob applied during the
            # PSUM drain by the DVE).
            for h in range(NH):
                for ci, (m0, ml) in enumerate(chunks):
                    y_ps = psum.tile([P, ml], f32, tag="y")
                    for it in range(NI):
                        nc.tensor.matmul(
                            y_ps[:],
                            wd_sb[:, it, h * P:(h + 1) * P],
                            acts[:, it, m0:m0 + ml],
                            start=(it == 0),
                            stop=(it == NI - 1),
                        )
                    y_sb = outs.tile([P, ml], f32, tag="y")
                    nc.vector.tensor_mul(y_sb[:], y_ps[:], pb_sb[:, m0:m0 + ml])
                    eng = nc.sync if (h * len(chunks) + ci) % 2 == 0 else nc.gpsimd
                    eng.dma_start(y.ap()[h, :, m0:m0 + ml], y_sb[:])

    nc.compile()
    return nc


def _pack_w(w_e):
    """[H, I] -> [NI, P, KH*P] bf16 with w[it, p, k*P+i] = w_e[k*P+p, it*P+i]."""
    w4 = w_e.reshape(KH, P, NI, P)
    return np.ascontiguousarray(
        w4.transpose(2, 1, 0, 3).reshape(NI, P, KH * P).astype(BF16)
    )


def kernel(hidden_states, gate_w, wg, wu, wd):
    global LAST_RESULTS
    x = np.ascontiguousarray(np.asarray(hidden_states, dtype=np.float32))
    gw = np.asarray(gate_w, dtype=np.float32)
    wg = np.asarray(wg, dtype=np.float32)
    wu = np.asarray(wu, dtype=np.float32)
    wd = np.asarray(wd, dtype=np.float32)

    # Router on host (part of the dispatch/sharding step).
    logits = np.tanh((x @ gw) / np.float32(SOFTCAP))
    top2 = np.argsort(-logits, axis=1, kind="stable")[:, :TOPK]  # [T, 2]
    v = np.take_along_axis(logits, top2, axis=1)                 # descending
    ex = np.exp(v - v[:, :1])
    pk = (ex / ex.sum(axis=1, keepdims=True)).astype(np.float32)  # [T, 2]

    token_ids, probs_e = [], []
    for e in range(E):
        mask = top2 == e
        rows = np.where(mask.any(axis=1))[0]
        kk = np.argmax(mask[rows], axis=1)
        token_ids.append(rows)
        probs_e.append(pk[rows, kk])

    n_max = max(len(r) for r in token_ids)
    M_PAD = max(64, -(-n_max // 8) * 8)

    nc = _compiled.get(M_PAD)
    if nc is None:
        nc = _build(M_PAD)
        _compiled[M_PAD] = nc

    in_maps = []
    for e in range(E):
        ids = token_ids[e]
        xe = np.zeros((M_PAD, H), np.float32)
        xe[: len(ids)] = x[ids]
        # [M_PAD, KH, P] -> [KH, P, M_PAD]
        xt_e = np.ascontiguousarray(
            xe.reshape(M_PAD, KH, P).transpose(1, 2, 0).astype(BF16)
        )
        pr = np.zeros((M_PAD,), np.float32)
        pr[: len(ids)] = probs_e[e]
        in_maps.append(
            {
                "xt": xt_e,
                "wg": _pack_w(wg[e]),
                "wu": _pack_w(wu[e]),
                "wd": np.ascontiguousarray(wd[e].astype(BF16)),
                "pb": np.ascontiguousarray(
                    np.broadcast_to(pr, (P, M_PAD)).copy()
                ),
            }
        )

    res = run_bass_kernel_spmd(nc, in_maps, core_ids=list(range(N_CORES)))
    LAST_RESULTS = res

    out = np.zeros((T, H), np.float32)
    for e in range(E):
        ids = token_ids[e]
        yT = res.results[e]["y"].reshape(H, M_PAD)          # [H, M_PAD]
        out[ids] += yT[:, : len(ids)].T
    return out


# revision 31
# speedup vs baseline: 1.0072x; 1.0005x over previous
"""Grok1-style MoE (T=2048, H=1024, E=8, I=2048, top-2) on 8 Trainium2 cores.

Strategy (expert-parallel, per the sharding hint):
  - Host: compute the tiny router (x @ gate_w, tanh softcap, top-2, softmax)
    and dispatch tokens by expert assignment (the "all-to-all dispatch" step:
    with full inputs on the host, dispatch = gather per expert), packing the
    per-core shards in bf16 device-friendly layouts.
  - Device (SPMD, 1 expert per core), all matmuls bf16 (1 row/cycle, half
    the HBM traffic and LDWEIGHTS cost of fp32r):
      phase 1: gT = wg_e^T x^T ; uT = wu_e^T x^T     ([I, M], fp32 PSUM)
               act = gelu_tanh(gT) * uT              (bf16 SBUF)
      phase 2: yT[h, m] = p[m] * sum_i wd_e[i, h] act[i, m]
               (wd stationary, act moving -> no M-padding to 128 needed;
               the router probs are applied by the DVE in fp32 during the
               PSUM->SBUF drain, against a host-replicated [P, M] prob tile)
  - Host: combine = scatter-add per-expert y^T into [T, H].
"""

import numpy as np
import ml_dtypes

import concourse.mybir as mybir
import concourse.tile as tile
from concourse import bacc
from concourse.bass_utils import run_bass_kernel_spmd

T, H, E, I_DIM, TOPK = 2048, 1024, 8, 2048, 2
SOFTCAP = 30.0
P = 128
N_CORES = 8
KH = H // P      # 8 contraction tiles (phase 1)
NI = I_DIM // P  # 16 i tiles
NH = H // P      # 8 h tiles (phase 2 output partition tiles)

BF16 = ml_dtypes.bfloat16

_compiled = {}
LAST_RESULTS = None


def _m_chunks(M_PAD):
    """Split [0, M_PAD) into near-equal chunks <= 512 (PSUM bank: 512 fp32)."""
    n_chunks = max(1, -(-M_PAD // 512))
    base = -(-(M_PAD // n_chunks) // 8) * 8
    chunks, off = [], 0
    for _ in range(n_chunks):
        ln = min(base, M_PAD - off)
        chunks.append((off, ln))
        off += ln
    return [c for c in chunks if c[1] > 0]


def _build(M_PAD):
    f32 = mybir.dt.float32
    bf16 = mybir.dt.bfloat16
    chunks = _m_chunks(M_PAD)

    nc = bacc.Bacc("TRN2", target_bir_lowering=False, num_devices=N_CORES)
    # Host-packed layouts (all DMAs contiguous per partition):
    #   xt  [KH, P, M_PAD] : xt[k, p, m] = x_e[m, k*P+p]            (bf16)
    #   wg  [NI, P, KH*P]  : wg[it, p, k*P+i] = wg_e[k*P+p, it*P+i] (bf16)
    #   wu  same as wg
    #   wd  [I, H]         : natural layout                          (bf16)
    #   pb  [P, M_PAD]     : router prob per token, partition-replicated
    #   y   [NH, P, M_PAD] : y[h, p, m] = out_e[m, h*P+p]            (fp32)
    xt = nc.dram_tensor("xt", [KH, P, M_PAD], bf16, kind="ExternalInput")
    wg = nc.dram_tensor("wg", [NI, P, KH * P], bf16, kind="ExternalInput")
    wu = nc.dram_tensor("wu", [NI, P, KH * P], bf16, kind="ExternalInput")
    wd = nc.dram_tensor("wd", [I_DIM, H], bf16, kind="ExternalInput")
    pb = nc.dram_tensor("pb", [P, M_PAD], f32, kind="ExternalInput")
    y = nc.dram_tensor("y", [NH, P, M_PAD], f32, kind="ExternalOutput")

    with tile.TileContext(nc) as tc:
        with (
            tc.tile_pool(name="persist", bufs=1) as persist,
            tc.tile_pool(name="wtiles", bufs=3) as wtiles,
            tc.tile_pool(name="gelu", bufs=2) as gpool,
            tc.tile_pool(name="outs", bufs=3) as outs,
            tc.tile_pool(name="psum", bufs=2, space="PSUM") as psum,
        ):
            xt_sb = persist.tile([P, KH, M_PAD], bf16)
            wd_sb = persist.tile([P, NI, H], bf16)
            pb_sb = persist.tile([P, M_PAD], f32)
            acts = persist.tile([P, NI, M_PAD], bf16)
            warm = persist.tile([P, 512], bf16)

            def w_src(w, it):
                return w.ap()[it].rearrange("p (ko i) -> p ko i", i=P)

            wg_sbs, wu_sbs = {}, {}

            def load_w(it):
                wg_sbs[it] = wtiles.tile([P, KH, P], bf16, tag="wg", name=f"wg{it}")
                wu_sbs[it] = wtiles.tile([P, KH, P], bf16, tag="wu", name=f"wu{it}")
                nc.gpsimd.dma_start(wg_sbs[it][:], w_src(wg, it))
                nc.scalar.dma_start(wu_sbs[it][:], w_src(wu, it))

            # Startup: each dma_start costs ~0.7us of sequencer issue time
            # and a ring streams ~200 GB/s, so use few, big pieces and put
            # the x k-pair pieces on TWO rings (sync + scalar, interleaved
            # with wu0) so the first i-tile's accumulation can start ~2us
            # earlier.  wg tiles stream on gpsimd, wu tiles on scalar.
            # Warm up the PE p-state during the initial DMA wait: ~12 junk
            # matmuls on a zeroed scratch tile keep the tensor engine busy
            # from the preamble until the first real operands land, so the
            # real matmuls start at full clock instead of ramping through
            # the 0.65/1.2 GHz p-states.
            nc.scalar.memzero(warm[:])
            w_ps = psum.tile([P, 512], f32, tag="g")
            for _ in range(12):
                nc.tensor.matmul(w_ps[:], warm[:, :P], warm[:], start=True, stop=True)

            wg_sbs[0] = wtiles.tile([P, KH, P], bf16, tag="wg", name="wg0")
            wu_sbs[0] = wtiles.tile([P, KH, P], bf16, tag="wu", name="wu0")
            nc.gpsimd.dma_start(wg_sbs[0][:], w_src(wg, 0))

            def x_piece(eng, k0, k1):
                eng.dma_start(
                    xt_sb[:, k0:k1],
                    xt.ap()[k0:k1].rearrange("k p m -> p k m"),
                )

            x_piece(nc.sync, 0, 2)
            x_piece(nc.scalar, 2, 4)
            x_piece(nc.sync, 4, 6)
            nc.scalar.dma_start(wu_sbs[0][:], w_src(wu, 0))
            x_piece(nc.gpsimd, 6, 8)
            load_w(1)

            # Phase 1: gT/uT = wg^T xT / wu^T xT per i-tile; act = gelu(g)*u.
            # wd tile loads and the prob tile interleave on the sync ring
            # mid-phase (consumed only in phase 2).
            for it in range(NI):
                if it + 2 < NI:
                    load_w(it + 2)
                if 4 <= it < 12:
                    for j in range(2):
                        wd_it = 2 * (it - 4) + j
                        nc.sync.dma_start(
                            wd_sb[:, wd_it], wd.ap()[wd_it * P:(wd_it + 1) * P, :]
                        )
                if it == 12:
                    nc.sync.dma_start(pb_sb[:], pb.ap())
                wg_sb, wu_sb = wg_sbs.pop(it), wu_sbs.pop(it)

                for (m0, ml) in chunks:
                    g_ps = psum.tile([P, ml], f32, tag="g")
                    for k in range(KH):
                        nc.tensor.matmul(
                            g_ps[:],
                            wg_sb[:, k],
                            xt_sb[:, k, m0:m0 + ml],
                            start=(k == 0),
                            stop=(k == KH - 1),
                        )
                    u_ps = psum.tile([P, ml], f32, tag="u")
                    for k in range(KH):
                        nc.tensor.matmul(
                            u_ps[:],
                            wu_sb[:, k],
                            xt_sb[:, k, m0:m0 + ml],
                            start=(k == 0),
                            stop=(k == KH - 1),
                        )
                    t = gpool.tile([P, ml], bf16, tag="t")
                    nc.scalar.activation(
                        t[:], g_ps[:], mybir.ActivationFunctionType.Gelu_apprx_tanh
                    )
                    nc.vector.tensor_mul(acts[:, it, m0:m0 + ml], t[:], u_ps[:])

            # Phase 2: yT[h, m] = p[m] * sum_i wd[i, h] * act[i, m]
            # (wd tile stationary, act moving; prob applied during the
            # PSUM drain by the DVE).
            for h in range(NH):
                for ci, (m0, ml) in enumerate(chunks):
                    y_ps = psum.tile([P, ml], f32, tag="y")
                    for it in range(NI):
                        nc.tensor.matmul(
                            y_ps[:],
                            wd_sb[:, it, h * P:(h + 1) * P],
                            acts[:, it, m0:m0 + ml],
                            start=(it == 0),
                            stop=(it == NI - 1),
                        )
                    y_sb = outs.tile([P, ml], f32, tag="y")
                    nc.vector.tensor_mul(y_sb[:], y_ps[:], pb_sb[:, m0:m0 + ml])
                    nc.sync.dma_start(y.ap()[h, :, m0:m0 + ml], y_sb[:])

    nc.compile()
    return nc


def _pack_w(w_e):
    """[H, I] -> [NI, P, KH*P] bf16 with w[it, p, k*P+i] = w_e[k*P+p, it*P+i]."""
    w4 = w_e.reshape(KH, P, NI, P)
    return np.ascontiguousarray(
        w4.transpose(2, 1, 0, 3).reshape(NI, P, KH * P).astype(BF16)
    )


def kernel(hidden_states, gate_w, wg, wu, wd):
    global LAST_RESULTS
    x = np.ascontiguousarray(np.asarray(hidden_states, dtype=np.float32))
    gw = np.asarray(gate_w, dtype=np.float32)
    wg = np.asarray(wg, dtype=np.float32)
    wu = np.asarray(wu, dtype=np.float32)
    wd = np.asarray(wd, dtype=np.float32)

    # Router on host (part of the dispatch/sharding step).
    logits = np.tanh((x @ gw) / np.float32(SOFTCAP))
    top2 = np.argsort(-logits, axis=1, kind="stable")[:, :TOPK]  # [T, 2]
    v = np.take_along_axis(logits, top2, axis=1)                 # descending
    ex = np.exp(v - v[:, :1])
    pk = (ex / ex.sum(axis=1, keepdims=True)).astype(np.float32)  # [T, 2]

    token_ids, probs_e = [], []
    for e in range(E):
        mask = top2 == e
        rows = np.where(mask.any(axis=1))[0]
        kk = np.argmax(mask[rows], axis=1)
        token_ids.append(rows)
        probs_e.append(pk[rows, kk])

    n_max = max(len(r) for r in token_ids)
    M_PAD = max(64, -(-n_max // 8) * 8)

    nc = _compiled.get(M_PAD)
    if nc is None:
        nc = _build(M_PAD)
        _compiled[M_PAD] = nc

    in_maps = []
    for e in range(E):
        ids = token_ids[e]
        xe = np.zeros((M_PAD, H), np.float32)
        xe[: len(ids)] = x[ids]
        # [M_PAD, KH, P] -> [KH, P, M_PAD]
        xt_e = np.ascontiguousarray(
            xe.reshape(M_PAD, KH, P).transpose(1, 2, 0).astype(BF16)
        )
        pr = np.zeros((M_PAD,), np.float32)
        pr[: len(ids)] = probs_e[e]
        in_maps.append(
            {
                "xt": xt_e,
                "wg": _pack_w(wg[e]),
                "wu": _pack_w(wu[e]),
                "wd": np.ascontiguousarray(wd[e].astype(BF16)),
                "pb": np.ascontiguousarray(
                    np.broadcast_to(pr, (P, M_PAD)).copy()
                ),
            }
        )

    res = run_bass_kernel_spmd(nc, in_maps, core_ids=list(range(N_CORES)))
    LAST_RESULTS = res

    out = np.zeros((T, H), np.float32)
    for e in range(E):
        ids = token_ids[e]
        yT = res.results[e]["y"].reshape(H, M_PAD)          # [H, M_PAD]
        out[ids] += yT[:, : len(ids)].T
    return out


# revision 32
# speedup vs baseline: 1.0156x; 1.0083x over previous
"""Grok1-style MoE (T=2048, H=1024, E=8, I=2048, top-2) on 8 Trainium2 cores.

Strategy (expert-parallel, per the sharding hint):
  - Host: compute the tiny router (x @ gate_w, tanh softcap, top-2, softmax)
    and dispatch tokens by expert assignment (the "all-to-all dispatch" step:
    with full inputs on the host, dispatch = gather per expert), packing the
    per-core shards in bf16 device-friendly layouts.
  - Device (SPMD, 1 expert per core), all matmuls bf16 (1 row/cycle, half
    the HBM traffic and LDWEIGHTS cost of fp32r):
      phase 1: gT = wg_e^T x^T ; uT = wu_e^T x^T     ([I, M], fp32 PSUM)
               act = gelu_tanh(gT) * uT              (bf16 SBUF)
      phase 2: yT[h, m] = p[m] * sum_i wd_e[i, h] act[i, m]
               (wd stationary, act moving -> no M-padding to 128 needed;
               the router probs are applied by the DVE in fp32 during the
               PSUM->SBUF drain, against a host-replicated [P, M] prob tile)
  - Host: combine = scatter-add per-expert y^T into [T, H].
"""

import numpy as np
import ml_dtypes

import concourse.mybir as mybir
import concourse.tile as tile
from concourse import bacc
from concourse.bass_utils import run_bass_kernel_spmd

T, H, E, I_DIM, TOPK = 2048, 1024, 8, 2048, 2
SOFTCAP = 30.0
P = 128
N_CORES = 8
KH = H // P      # 8 contraction tiles (phase 1)
NI = I_DIM // P  # 16 i tiles
NH = H // P      # 8 h tiles (phase 2 output partition tiles)

BF16 = ml_dtypes.bfloat16

_compiled = {}
LAST_RESULTS = None


def _m_chunks(M_PAD):
    """Split [0, M_PAD) into near-equal chunks <= 512 (PSUM bank: 512 fp32)."""
    n_chunks = max(1, -(-M_PAD // 512))
    base = -(-(M_PAD // n_chunks) // 8) * 8
    chunks, off = [], 0
    for _ in range(n_chunks):
        ln = min(base, M_PAD - off)
        chunks.append((off, ln))
        off += ln
    return [c for c in chunks if c[1] > 0]


def _build(M_PAD):
    f32 = mybir.dt.float32
    bf16 = mybir.dt.bfloat16
    chunks = _m_chunks(M_PAD)

    nc = bacc.Bacc("TRN2", target_bir_lowering=False, num_devices=N_CORES)
    # Host-packed layouts (all DMAs contiguous per partition):
    #   xt  [KH, P, M_PAD] : xt[k, p, m] = x_e[m, k*P+p]            (bf16)
    #   wg  [NI, P, KH*P]  : wg[it, p, k*P+i] = wg_e[k*P+p, it*P+i] (bf16)
    #   wu  same as wg
    #   wd  [I, H]         : natural layout                          (bf16)
    #   pb  [P, M_PAD]     : router prob per token, partition-replicated
    #   y   [NH, P, M_PAD] : y[h, p, m] = out_e[m, h*P+p]            (fp32)
    xt = nc.dram_tensor("xt", [KH, P, M_PAD], bf16, kind="ExternalInput")
    wg = nc.dram_tensor("wg", [NI, P, KH * P], bf16, kind="ExternalInput")
    wu = nc.dram_tensor("wu", [NI, P, KH * P], bf16, kind="ExternalInput")
    wd = nc.dram_tensor("wd", [I_DIM, H], bf16, kind="ExternalInput")
    pb = nc.dram_tensor("pb", [P, M_PAD], f32, kind="ExternalInput")
    y = nc.dram_tensor("y", [NH, P, M_PAD], f32, kind="ExternalOutput")

    with tile.TileContext(nc) as tc:
        with (
            tc.tile_pool(name="persist", bufs=1) as persist,
            tc.tile_pool(name="wtiles", bufs=3) as wtiles,
            tc.tile_pool(name="gelu", bufs=2) as gpool,
            tc.tile_pool(name="outs", bufs=3) as outs,
            tc.tile_pool(name="psum", bufs=2, space="PSUM") as psum,
        ):
            xt_sb = persist.tile([P, KH, M_PAD], bf16)
            wd_sb = persist.tile([P, NI, H], bf16)
            pb_sb = persist.tile([P, M_PAD], f32)
            acts = persist.tile([P, NI, M_PAD], bf16)
            warm = persist.tile([P, 512], bf16)

            def w_src(w, it):
                return w.ap()[it].rearrange("p (ko i) -> p ko i", i=P)

            wg_sbs, wu_sbs = {}, {}

            def load_w(it):
                wg_sbs[it] = wtiles.tile([P, KH, P], bf16, tag="wg", name=f"wg{it}")
                wu_sbs[it] = wtiles.tile([P, KH, P], bf16, tag="wu", name=f"wu{it}")
                nc.gpsimd.dma_start(wg_sbs[it][:], w_src(wg, it))
                nc.scalar.dma_start(wu_sbs[it][:], w_src(wu, it))

            # Startup: each dma_start costs ~0.7us of sequencer issue time
            # and a ring streams ~200 GB/s, so use few, big pieces and put
            # the x k-pair pieces on TWO rings (sync + scalar, interleaved
            # with wu0) so the first i-tile's accumulation can start ~2us
            # earlier.  wg tiles stream on gpsimd, wu tiles on scalar.
            # Warm up the PE p-state during the initial DMA wait: ~12 junk
            # matmuls on a zeroed scratch tile keep the tensor engine busy
            # from the preamble until the first real operands land, so the
            # real matmuls start at full clock instead of ramping through
            # the 0.65/1.2 GHz p-states.
            nc.scalar.memzero(warm[:])
            w_ps = psum.tile([P, 512], f32, tag="g")
            for _ in range(12):
                nc.tensor.matmul(w_ps[:], warm[:, :P], warm[:], start=True, stop=True)

            wg_sbs[0] = wtiles.tile([P, KH, P], bf16, tag="wg", name="wg0")
            wu_sbs[0] = wtiles.tile([P, KH, P], bf16, tag="wu", name="wu0")
            nc.gpsimd.dma_start(wg_sbs[0][:], w_src(wg, 0))

            def x_piece(eng, k0, k1):
                eng.dma_start(
                    xt_sb[:, k0:k1],
                    xt.ap()[k0:k1].rearrange("k p m -> p k m"),
                )

            x_piece(nc.sync, 0, 2)
            x_piece(nc.scalar, 2, 4)
            x_piece(nc.sync, 4, 6)
            nc.scalar.dma_start(wu_sbs[0][:], w_src(wu, 0))
            x_piece(nc.gpsimd, 6, 8)
            load_w(1)

            # Phase 1: gT/uT = wg^T xT / wu^T xT per i-tile; act = gelu(g)*u.
            # wd tile loads and the prob tile interleave on the sync ring
            # mid-phase (consumed only in phase 2).
            for it in range(NI):
                if it + 2 < NI:
                    load_w(it + 2)
                if 4 <= it < 12:
                    for j in range(2):
                        wd_it = 2 * (it - 4) + j
                        nc.sync.dma_start(
                            wd_sb[:, wd_it], wd.ap()[wd_it * P:(wd_it + 1) * P, :]
                        )
                if it == 12:
                    nc.sync.dma_start(pb_sb[:], pb.ap())
                wg_sb, wu_sb = wg_sbs.pop(it), wu_sbs.pop(it)

                for (m0, ml) in chunks:
                    g_ps = psum.tile([P, ml], f32, tag="g")
                    for k in range(KH):
                        nc.tensor.matmul(
                            g_ps[:],
                            wg_sb[:, k],
                            xt_sb[:, k, m0:m0 + ml],
                            start=(k == 0),
                            stop=(k == KH - 1),
                        )
                    u_ps = psum.tile([P, ml], f32, tag="u")
                    for k in range(KH):
                        nc.tensor.matmul(
                            u_ps[:],
                            wu_sb[:, k],
                            xt_sb[:, k, m0:m0 + ml],
                            start=(k == 0),
                            stop=(k == KH - 1),
                        )
                    t = gpool.tile([P, ml], bf16, tag="t")
                    nc.scalar.activation(
                        t[:], g_ps[:], mybir.ActivationFunctionType.Gelu_apprx_tanh
                    )
                    nc.vector.tensor_mul(acts[:, it, m0:m0 + ml], t[:], u_ps[:])

            # Phase 2: yT[h, m] = p[m] * sum_i wd[i, h] * act[i, m]
            # (wd tile stationary, act moving; prob applied during the
            # PSUM drain by the DVE).
            def _half(c):
                m0, ml = c
                h1 = ml // 2 // 8 * 8
                return [(m0, h1), (m0 + h1, ml - h1)]

            for h in range(NH):
                # Final h-tile: halve the chunks so the post-matmul drain
                # (prob-mul + DMA) after the very last matmul is shorter.
                p2_chunks = chunks if h < NH - 1 else [
                    cc for c in chunks for cc in _half(c)
                ]
                for ci, (m0, ml) in enumerate(p2_chunks):
                    y_ps = psum.tile([P, ml], f32, tag="y")
                    for it in range(NI):
                        nc.tensor.matmul(
                            y_ps[:],
                            wd_sb[:, it, h * P:(h + 1) * P],
                            acts[:, it, m0:m0 + ml],
                            start=(it == 0),
                            stop=(it == NI - 1),
                        )
                    y_sb = outs.tile([P, ml], f32, tag="y")
                    nc.vector.tensor_mul(y_sb[:], y_ps[:], pb_sb[:, m0:m0 + ml])
                    nc.sync.dma_start(y.ap()[h, :, m0:m0 + ml], y_sb[:])

    nc.compile()
    return nc


def _pack_w(w_e):
    """[H, I] -> [NI, P, KH*P] bf16 with w[it, p, k*P+i] = w_e[k*P+p, it*P+i]."""
    w4 = w_e.reshape(KH, P, NI, P)
    return np.ascontiguousarray(
        w4.transpose(2, 1, 0, 3).reshape(NI, P, KH * P).astype(BF16)
    )


def kernel(hidden_states, gate_w, wg, wu, wd):
    global LAST_RESULTS
    x = np.ascontiguousarray(np.asarray(hidden_states, dtype=np.float32))
    gw = np.asarray(gate_w, dtype=np.float32)
    wg = np.asarray(wg, dtype=np.float32)
    wu = np.asarray(wu, dtype=np.float32)
    wd = np.asarray(wd, dtype=np.float32)

    # Router on host (part of the dispatch/sharding step).
    logits = np.tanh((x @ gw) / np.float32(SOFTCAP))
    top2 = np.argsort(-logits, axis=1, kind="stable")[:, :TOPK]  # [T, 2]
    v = np.take_along_axis(logits, top2, axis=1)                 # descending
    ex = np.exp(v - v[:, :1])
    pk = (ex / ex.sum(axis=1, keepdims=True)).astype(np.float32)  # [T, 2]

    token_ids, probs_e = [], []
    for e in range(E):
        mask = top2 == e
        rows = np.where(mask.any(axis=1))[0]
        kk = np.argmax(mask[rows], axis=1)
        token_ids.append(rows)
        probs_e.append(pk[rows, kk])

    n_max = max(len(r) for r in token_ids)
    M_PAD = max(64, -(-n_max // 8) * 8)

    nc = _compiled.get(M_PAD)
    if nc is None:
        nc = _build(M_PAD)
        _compiled[M_PAD] = nc

    in_maps = []
    for e in range(E):
        ids = token_ids[e]
        xe = np.zeros((M_PAD, H), np.float32)
        xe[: len(ids)] = x[ids]
        # [M_PAD, KH, P] -> [KH, P, M_PAD]
        xt_e = np.ascontiguousarray(
            xe.reshape(M_PAD, KH, P).transpose(1, 2, 0).astype(BF16)
        )
        pr = np.zeros((M_PAD,), np.float32)
        pr[: len(ids)] = probs_e[e]
        in_maps.append(
            {
                "xt": xt_e,
                "wg": _pack_w(wg[e]),
                "wu": _pack_w(wu[e]),
                "wd": np.ascontiguousarray(wd[e].astype(BF16)),
                "pb": np.ascontiguousarray(
                    np.broadcast_to(pr, (P, M_PAD)).copy()
                ),
            }
        )

    res = run_bass_kernel_spmd(nc, in_maps, core_ids=list(range(N_CORES)))
    LAST_RESULTS = res

    out = np.zeros((T, H), np.float32)
    for e in range(E):
        ids = token_ids[e]
        yT = res.results[e]["y"].reshape(H, M_PAD)          # [H, M_PAD]
        out[ids] += yT[:, : len(ids)].T
    return out
